# revision 5
# baseline (speedup 1.0000x reference)
"""APLoss distributed Bass kernel for 8 TRN2 NeuronCores.

Math (reference, restructured):
    sur[i,j] = relu(MARGIN - f_i + y_j)^2        (P=2048 rows, N=16384 cols)
    S_i = sum_j sur[i,j];  T_i = sum_j sur[i,j]*mask[j]
    ua_i = (1-g)*u_all[index_p[i]] + g*S_i/N
    up_i = (1-g)*u_pos[index_p[i]] + g*T_i/N
    loss = sum_i (up_i*S_i - ua_i*T_i) / ua_i^2 / (P*N)

Sharding: rows (positives) split 8 ways, 256 rows/core; y replicated.
Per-core device layout: columns j on partitions (128 j-blocks of 128),
rows i on the free axis (256).  Pipeline per core:
  DVE:  r = relu(negf_bcast + y_j)   (tensor_scalar add+max, per j-block)
  DVE/ACT split: z = r*r             (grouped long-FD square pass)
  PE:   ones^T @ z_block -> PSUM accumulators T (masked cols) / R (rest)
  combine on [1,256] vectors -> per-core scalar partial; host sums cores.
"""

import sys

if "/opt/trn_rl_repo" not in sys.path:
    sys.path.insert(0, "/opt/trn_rl_repo")

import numpy as np

import concourse.bass as bass
import concourse.tile as tile
from concourse import bacc, mybir
from concourse import bass_utils

N = 16384
P = 2048
N_CORES = 8
PC = P // N_CORES          # rows per core (free dim)
JB = 128                   # j-block size (partitions)
NB = N // JB               # number of j-blocks
GAMMA = 0.99
MARGIN = 1.0
INV_PN = 1.0 / (P * N)     # 2^-25, exact

SQ_GROUP = 8               # j-blocks per square instruction (FD = SQ_GROUP*PC)
ACT_SQ_FRACTION = 0.625    # fraction of square groups on the scalar engine

TRACE = False              # set True (e.g. from test.py) to capture HW timing
LAST_RESULT = None         # BassKernelResults of the most recent run

_COMPILED = {}             # cache: block classification -> compiled Bacc

f32 = mybir.dt.float32
bf16 = mybir.dt.bfloat16
Alu = mybir.AluOpType
Act = mybir.ActivationFunctionType


def _classify_blocks(mask: np.ndarray):
    """Per j-block: 'T' (all masked), 'R' (none masked), or 'M' (mixed)."""
    blocks = []
    for b in range(NB):
        mb = mask[b * JB:(b + 1) * JB]
        if mb.all():
            blocks.append("T")
        elif not mb.any():
            blocks.append("R")
        else:
            blocks.append("M")
    return tuple(blocks)


def _build(blockclass):
    nc = bacc.Bacc("TRN2", target_bir_lowering=False, debug=False,
                   num_devices=N_CORES)

    yb_d = nc.dram_tensor("yb", [JB, NB], f32, kind="ExternalInput")
    maskb_d = nc.dram_tensor("maskb", [JB, NB], f32, kind="ExternalInput")
    negf_d = nc.dram_tensor("negf", [JB, PC], f32, kind="ExternalInput")
    uall_d = nc.dram_tensor("uall", [1, PC], f32, kind="ExternalInput")
    upos_d = nc.dram_tensor("upos", [1, PC], f32, kind="ExternalInput")
    out_d = nc.dram_tensor("out", [1, 1], f32, kind="ExternalOutput")

    n_groups = NB // SQ_GROUP
    n_act_groups = int(round(n_groups * ACT_SQ_FRACTION))

    has_T = any(c in ("T", "M") for c in blockclass)
    has_R = any(c in ("R", "M") for c in blockclass)

    with tile.TileContext(nc) as tc:
        with (
            tc.tile_pool(name="const", bufs=1) as cpool,
            tc.tile_pool(name="big", bufs=1) as bpool,
            tc.tile_pool(name="psum", bufs=1, space="PSUM") as ppool,
            tc.tile_pool(name="small", bufs=1) as spool,
        ):
            y_f32 = cpool.tile([JB, NB], f32, tag="y_f32")
            nc.sync.dma_start(y_f32[:], yb_d[:])

            mask_f32 = cpool.tile([JB, NB], f32, tag="mask_f32")
            nc.sync.dma_start(mask_f32[:], maskb_d[:])
            mask_bf = cpool.tile([JB, NB], bf16, tag="mask_bf")
            nc.vector.tensor_copy(mask_bf[:], mask_f32[:])
            # 1-mask for the R side of mixed blocks
            imask_bf = cpool.tile([JB, NB], bf16, tag="imask_bf")
            nc.vector.tensor_scalar(imask_bf[:], mask_f32[:], -1.0, 1.0,
                                    Alu.mult, Alu.add)

            negf_f32 = cpool.tile([JB, PC], f32, tag="negf_f32")
            nc.sync.dma_start(negf_f32[:], negf_d[:])
            negf_bf = cpool.tile([JB, PC], bf16, tag="negf_bf")
            nc.vector.tensor_copy(negf_bf[:], negf_f32[:])

            ones_bf = cpool.tile([JB, 1], bf16, tag="ones_bf")
            nc.vector.memset(ones_bf[:], 1.0)

            uall_s = spool.tile([1, PC], f32, tag="uall_s")
            nc.sync.dma_start(uall_s[:], uall_d[:])
            upos_s = spool.tile([1, PC], f32, tag="upos_s")
            nc.sync.dma_start(upos_s[:], upos_d[:])

            r_all = bpool.tile([JB, NB * PC], bf16, tag="r_all")
            z_all = bpool.tile([JB, NB * PC], bf16, tag="z_all")

            # ---- relu pass (DVE): r_b = relu(negf + y_b) ----
            for b in range(NB):
                nc.vector.tensor_scalar(
                    r_all[:, b * PC:(b + 1) * PC], negf_bf[:],
                    y_f32[:, b:b + 1], 0.0, Alu.add, Alu.max)

            # ---- square pass (split ACT/DVE): z = r*r ----
            for g in range(n_groups):
                sl = slice(g * SQ_GROUP * PC, (g + 1) * SQ_GROUP * PC)
                if g % n_groups < n_act_groups:
                    nc.scalar.activation(z_all[:, sl], r_all[:, sl], Act.Square)
                else:
                    nc.vector.tensor_mul(z_all[:, sl], r_all[:, sl],
                                         r_all[:, sl])

            # ---- PE reduction: acc_T/acc_R[0, i] += sum_j z[j, i] ----
            acc_T = ppool.tile([1, PC], f32, name="acc_T", tag="acc_T") if has_T else None
            acc_R = ppool.tile([1, PC], f32, name="acc_R", tag="acc_R") if has_R else None
            t_mms = []
            r_mms = []
            for b, cls in enumerate(blockclass):
                zb = z_all[:, b * PC:(b + 1) * PC]
                if cls == "T":
                    t_mms.append((ones_bf[:, 0:1], zb))
                elif cls == "R":
                    r_mms.append((ones_bf[:, 0:1], zb))
                else:  # mixed: mask-weighted into T, (1-mask) into R
                    t_mms.append((mask_bf[:, b:b + 1], zb))
                    r_mms.append((imask_bf[:, b:b + 1], zb))
            for i, (w, zb) in enumerate(t_mms):
                nc.tensor.matmul(acc_T[:], w, zb, start=(i == 0),
                                 stop=(i == len(t_mms) - 1))
            for i, (w, zb) in enumerate(r_mms):
                nc.tensor.matmul(acc_R[:], w, zb, start=(i == 0),
                                 stop=(i == len(r_mms) - 1))

            # ---- combine on [1, PC] vectors (partition 0) ----
            T_s = spool.tile([1, PC], f32, tag="T_s")
            if acc_T is not None:
                nc.vector.tensor_copy(T_s[:], acc_T[:])
            else:
                nc.vector.memset(T_s[:], 0.0)
            R_s = spool.tile([1, PC], f32, tag="R_s")
            if acc_R is not None:
                nc.vector.tensor_copy(R_s[:], acc_R[:])
            else:
                nc.vector.memset(R_s[:], 0.0)

            S_s = spool.tile([1, PC], f32, tag="S_s")
            nc.vector.tensor_add(S_s[:], T_s[:], R_s[:])

            # ua = (1-g)*uall + (g/N)*S ; up = (1-g)*upos + (g/N)*T
            ua = spool.tile([1, PC], f32, tag="ua")
            nc.vector.tensor_scalar(ua[:], uall_s[:], 1.0 - GAMMA, 0.0,
                                    Alu.mult, Alu.add)
            nc.vector.scalar_tensor_tensor(ua[:], S_s[:], GAMMA / N, ua[:],
                                           Alu.mult, Alu.add)
            up = spool.tile([1, PC], f32, tag="up")
            nc.vector.tensor_scalar(up[:], upos_s[:], 1.0 - GAMMA, 0.0,
                                    Alu.mult, Alu.add)
            nc.vector.scalar_tensor_tensor(up[:], T_s[:], GAMMA / N, up[:],
                                           Alu.mult, Alu.add)

            inv = spool.tile([1, PC], f32, tag="inv")
            nc.vector.reciprocal(inv[:], ua[:])

            t1 = spool.tile([1, PC], f32, tag="t1")
            nc.vector.tensor_mul(t1[:], up[:], S_s[:])
            t2 = spool.tile([1, PC], f32, tag="t2")
            nc.vector.tensor_mul(t2[:], ua[:], T_s[:])
            d = spool.tile([1, PC], f32, tag="d")
            nc.vector.tensor_sub(d[:], t1[:], t2[:])
            nc.vector.tensor_mul(d[:], d[:], inv[:])
            nc.vector.tensor_mul(d[:], d[:], inv[:])

            dummy = spool.tile([1, PC], f32, tag="dummy")
            partial = spool.tile([1, 1], f32, tag="partial")
            nc.vector.tensor_scalar(dummy[:], d[:], INV_PN, 0.0, Alu.mult,
                                    Alu.add, accum_out=partial[:])
            nc.sync.dma_start(out_d[:], partial[:])

    nc.compile()
    return nc


def kernel(y_pred, y_true, index_p, pos_idx, u_all, u_pos):
    global LAST_RESULT

    yp = np.asarray(y_pred, dtype=np.float32).reshape(-1)
    mask = (np.asarray(y_true, dtype=np.float32).reshape(-1) == 1.0)
    index_p = np.asarray(index_p).reshape(-1)
    pos_idx = np.asarray(pos_idx).reshape(-1)
    u_all_b = np.asarray(u_all, dtype=np.float32).reshape(-1)[index_p]
    u_pos_b = np.asarray(u_pos, dtype=np.float32).reshape(-1)[index_p]

    f_ps = yp[pos_idx]                        # (P,)
    negf = (MARGIN - f_ps).astype(np.float32)  # (P,)

    blockclass = _classify_blocks(mask)
    nc = _COMPILED.get(blockclass)
    if nc is None:
        nc = _build(blockclass)
        _COMPILED[blockclass] = nc

    # y rearranged: yb[p, b] = yp[b*JB + p]
    yb = np.ascontiguousarray(yp.reshape(NB, JB).T)
    maskb = np.ascontiguousarray(
        mask.astype(np.float32).reshape(NB, JB).T)

    in_maps = []
    for c in range(N_CORES):
        rs = slice(c * PC, (c + 1) * PC)
        in_maps.append({
            "yb": yb,
            "maskb": maskb,
            "negf": np.ascontiguousarray(
                np.broadcast_to(negf[rs], (JB, PC))).astype(np.float32),
            "uall": u_all_b[rs].reshape(1, PC).astype(np.float32),
            "upos": u_pos_b[rs].reshape(1, PC).astype(np.float32),
        })

    res = bass_utils.run_bass_kernel_spmd(
        nc, in_maps, core_ids=list(range(N_CORES)), trace=TRACE)
    LAST_RESULT = res

    total = np.float32(0.0)
    for c in range(N_CORES):
        total = np.float32(total + res.results[c]["out"][0, 0])
    return np.asarray(total, dtype=np.float32)


# revision 6
# speedup vs baseline: 1.4635x; 1.4635x over previous
"""APLoss distributed Bass kernel for 8 TRN2 NeuronCores.

Reference math, restructured with an indicator decomposition:
    sur[i,j] = relu(t)^2,  t = negf_i + y_j,  negf_i = MARGIN - f_i
    relu(t)^2 = t^2 * H,   H = 1[t > 0]
    S_i = sum_j sur = negf_i^2 * A_i + 2*negf_i * B_i + C_i
      where A_i = sum_j H_ij, B_i = sum_j H_ij*y_j, C_i = sum_j H_ij*y_j^2
    T_i = masked version with (Am, Bm, Cm) using weights m_j*[1, y, y^2]
    ua_i = (1-g)*u_all[index_p[i]] + g*S_i/N
    up_i = (1-g)*u_pos[index_p[i]] + g*T_i/N
    loss = sum_i (up_i*S_i - ua_i*T_i) / ua_i^2 / (P*N)

Sharding: rows (positives) split 8 ways, 256 rows/core; y replicated.
Device layout: columns j on partitions (128 j-blocks of 128), rows i on
the free axis (256). Per core:
  DVE  (93 blocks): H = (negf + y_j) > 0           (tensor_scalar add,is_gt)
  ACT  (35 blocks): Hs = Sign(negf + y_j)          (activation, bias=y_j)
  PE: W_b^T @ H_b -> psumH[6,256]; W_b^T @ Hs_b -> psumS[6,256]
      with W_b = [1, y, y^2, m, m*y, m*y^2] per block (6-col stationary)
  Sign-block sums are corrected on device: H.W = (Hs.W + sum(W))/2, with
  sum(W) over the ACT column range passed as host constants.
  Finalize transposed to [128,2] (rows on partitions) for cheap vector ops;
  per-core scalar partial out; host sums the 8 partials.
"""

import sys

if "/opt/trn_rl_repo" not in sys.path:
    sys.path.insert(0, "/opt/trn_rl_repo")

import numpy as np

import concourse.bass as bass
import concourse.tile as tile
from concourse import bacc, mybir
from concourse import bass_utils
from concourse.masks import make_identity

N = 16384
P = 2048
N_CORES = 8
PC = P // N_CORES          # rows per core (free dim)
JB = 128                   # j-block size (partitions)
NB = N // JB               # number of j-blocks
NH = PC // JB              # halves of the row range (2)
GAMMA = 0.99
MARGIN = 1.0
INV_PN = 1.0 / (P * N)     # 2^-25, exact

DVE_BLOCKS = 93            # H-blocks on the vector engine; rest on scalar

TRACE = False
LAST_RESULT = None

_COMPILED = {}

f32 = mybir.dt.float32
bf16 = mybir.dt.bfloat16
Alu = mybir.AluOpType
Act = mybir.ActivationFunctionType


def _build():
    nc = bacc.Bacc("TRN2", target_bir_lowering=False, debug=False,
                   num_devices=N_CORES)

    yb_d = nc.dram_tensor("yb", [JB, NB], f32, kind="ExternalInput")
    maskb_d = nc.dram_tensor("maskb", [JB, NB], f32, kind="ExternalInput")
    negf_d = nc.dram_tensor("negf", [JB, PC], f32, kind="ExternalInput")
    negfT_d = nc.dram_tensor("negfT", [JB, NH], f32, kind="ExternalInput")
    uallT_d = nc.dram_tensor("uallT", [JB, NH], f32, kind="ExternalInput")
    uposT_d = nc.dram_tensor("uposT", [JB, NH], f32, kind="ExternalInput")
    corrb_d = nc.dram_tensor("corrb", [JB, 6], f32, kind="ExternalInput")
    out_d = nc.dram_tensor("out", [1, 1], f32, kind="ExternalOutput")

    with tile.TileContext(nc) as tc:
        with (
            tc.tile_pool(name="const", bufs=1) as cpool,
            tc.tile_pool(name="hpool", bufs=NB) as hpool,
            tc.tile_pool(name="psum", bufs=1, space="PSUM") as ppool,
            tc.tile_pool(name="small", bufs=1) as spool,
        ):
            y_f32 = cpool.tile([JB, NB], f32, name="y_f32")
            nc.sync.dma_start(y_f32[:], yb_d[:])
            mask_f32 = cpool.tile([JB, NB], f32, name="mask_f32")
            nc.sync.dma_start(mask_f32[:], maskb_d[:])
            negf_f32 = cpool.tile([JB, PC], f32, name="negf_f32")
            nc.sync.dma_start(negf_f32[:], negf_d[:])
            negf_bf = cpool.tile([JB, PC], bf16, name="negf_bf")
            nc.vector.tensor_copy(negf_bf[:], negf_f32[:])

            negfT = spool.tile([JB, NH], f32, name="negfT")
            nc.sync.dma_start(negfT[:], negfT_d[:])
            uallT = spool.tile([JB, NH], f32, name="uallT")
            nc.sync.dma_start(uallT[:], uallT_d[:])
            uposT = spool.tile([JB, NH], f32, name="uposT")
            nc.sync.dma_start(uposT[:], uposT_d[:])
            corrb = spool.tile([JB, 6], f32, name="corrb")
            nc.sync.dma_start(corrb[:], corrb_d[:])

            ident = cpool.tile([JB, JB], f32, name="ident")
            make_identity(nc, ident)
            ones_f = cpool.tile([JB, 1], f32, name="ones_f")
            nc.vector.memset(ones_f[:], 1.0)

            # ---- per-block 6-column stationary W = [1,y,y^2,m,my,my^2] ----
            W_all = cpool.tile([JB, NB * 6], bf16, name="W_all")
            w = W_all[:].rearrange("p (b k) -> p b k", k=6)
            nc.vector.tensor_scalar(w[:, :, 0], y_f32[:], 0.0, 1.0,
                                    Alu.mult, Alu.add)
            nc.vector.tensor_copy(w[:, :, 1], y_f32[:])
            nc.vector.tensor_mul(w[:, :, 2], y_f32[:], y_f32[:])
            nc.vector.tensor_copy(w[:, :, 3], mask_f32[:])
            nc.vector.tensor_mul(w[:, :, 4], mask_f32[:], y_f32[:])
            nc.vector.tensor_mul(w[:, :, 5], w[:, :, 4], y_f32[:])

            # ---- H pass ----
            h_tiles = []
            for b in range(NB):
                h = hpool.tile([JB, PC], bf16, name=f"h{b}", tag="h")
                if b < DVE_BLOCKS:
                    nc.vector.tensor_scalar(h[:], negf_bf[:],
                                            y_f32[:, b:b + 1], 0.0,
                                            Alu.add, Alu.is_gt)
                else:
                    nc.scalar.activation(h[:], negf_bf[:], Act.Sign,
                                         bias=y_f32[:, b:b + 1])
                h_tiles.append(h)

            # ---- PE contraction ----
            psumH = ppool.tile([6, PC], f32, name="psumH")
            psumS = ppool.tile([6, PC], f32, name="psumS")
            for b in range(NB):
                acc = psumH if b < DVE_BLOCKS else psumS
                first = b == 0 or b == DVE_BLOCKS
                last = b == DVE_BLOCKS - 1 or b == NB - 1
                nc.tensor.matmul(acc[:], W_all[:, b * 6:(b + 1) * 6],
                                 h_tiles[b][:], start=first, stop=last)

            # ---- transpose [6,256] sums to [128,6] per half ----
            Hsb = spool.tile([6, PC], f32, name="Hsb")
            nc.vector.tensor_copy(Hsb[:], psumH[:])
            Ssb = spool.tile([6, PC], f32, name="Ssb")
            nc.vector.tensor_copy(Ssb[:], psumS[:])

            contrib = spool.tile([JB, NH], f32, name="contrib")
            psumT = ppool.tile([JB, NH * 6], f32, name="psumT")
            psumT2 = ppool.tile([JB, NH * 6], f32, name="psumT2")
            for hh in range(NH):
                nc.tensor.transpose(psumT[:, hh * 6:(hh + 1) * 6],
                                    Hsb[:, hh * JB:(hh + 1) * JB],
                                    ident[0:6, 0:6])
                nc.tensor.transpose(psumT2[:, hh * 6:(hh + 1) * 6],
                                    Ssb[:, hh * JB:(hh + 1) * JB],
                                    ident[0:6, 0:6])

            for hh in range(NH):
                VH = spool.tile([JB, 6], f32, name=f"VH{hh}", tag="VH")
                nc.vector.tensor_copy(VH[:], psumT[:, hh * 6:(hh + 1) * 6])
                VS = spool.tile([JB, 6], f32, name=f"VS{hh}", tag="VS")
                nc.vector.tensor_copy(VS[:], psumT2[:, hh * 6:(hh + 1) * 6])

                # Vc = VH + 0.5*(VS + corr)  -> [A,B,C,Am,Bm,Cm]
                Vc = spool.tile([JB, 6], f32, name=f"Vc{hh}", tag="Vc")
                nc.vector.tensor_add(Vc[:], VS[:], corrb[:])
                nc.vector.scalar_tensor_tensor(Vc[:], Vc[:], 0.5, VH[:],
                                               Alu.mult, Alu.add)

                nf = negfT[:, hh:hh + 1]
                nf2 = spool.tile([JB, 1], f32, name=f"nf2{hh}", tag="nf2")
                nc.vector.tensor_mul(nf2[:], nf, nf)
                nf_2 = spool.tile([JB, 1], f32, name=f"nf_2{hh}", tag="nf_2")
                nc.vector.tensor_scalar(nf_2[:], nf, 2.0, 0.0,
                                        Alu.mult, Alu.add)

                # S = negf^2*A + (2negf*B + C); T likewise on masked cols
                S = spool.tile([JB, 1], f32, name=f"S{hh}", tag="S")
                nc.vector.scalar_tensor_tensor(S[:], Vc[:, 1:2], nf_2[:],
                                               Vc[:, 2:3], Alu.mult, Alu.add)
                nc.vector.scalar_tensor_tensor(S[:], Vc[:, 0:1], nf2[:],
                                               S[:], Alu.mult, Alu.add)
                T = spool.tile([JB, 1], f32, name=f"T{hh}", tag="T")
                nc.vector.scalar_tensor_tensor(T[:], Vc[:, 4:5], nf_2[:],
                                               Vc[:, 5:6], Alu.mult, Alu.add)
                nc.vector.scalar_tensor_tensor(T[:], Vc[:, 3:4], nf2[:],
                                               T[:], Alu.mult, Alu.add)

                # ua = (1-g)*uall + (g/N)*S ; up = (1-g)*upos + (g/N)*T
                ua = spool.tile([JB, 1], f32, name=f"ua{hh}", tag="ua")
                nc.vector.tensor_scalar(ua[:], uallT[:, hh:hh + 1],
                                        1.0 - GAMMA, 0.0, Alu.mult, Alu.add)
                nc.vector.scalar_tensor_tensor(ua[:], S[:], GAMMA / N, ua[:],
                                               Alu.mult, Alu.add)
                up = spool.tile([JB, 1], f32, name=f"up{hh}", tag="up")
                nc.vector.tensor_scalar(up[:], uposT[:, hh:hh + 1],
                                        1.0 - GAMMA, 0.0, Alu.mult, Alu.add)
                nc.vector.scalar_tensor_tensor(up[:], T[:], GAMMA / N, up[:],
                                               Alu.mult, Alu.add)

                inv = spool.tile([JB, 1], f32, name=f"inv{hh}", tag="inv")
                nc.vector.reciprocal(inv[:], ua[:])

                t1 = spool.tile([JB, 1], f32, name=f"t1{hh}", tag="t1")
                nc.vector.tensor_mul(t1[:], up[:], S[:])
                t2 = spool.tile([JB, 1], f32, name=f"t2{hh}", tag="t2")
                nc.vector.tensor_mul(t2[:], ua[:], T[:])
                d = spool.tile([JB, 1], f32, name=f"d{hh}", tag="d")
                nc.vector.tensor_sub(d[:], t1[:], t2[:])
                nc.vector.tensor_mul(d[:], d[:], inv[:])
                nc.vector.tensor_mul(d[:], d[:], inv[:])
                nc.vector.tensor_copy(contrib[:, hh:hh + 1], d[:])

            csum = spool.tile([JB, 1], f32, name="csum")
            nc.vector.tensor_add(csum[:], contrib[:, 0:1], contrib[:, 1:2])
            psum1 = ppool.tile([1, 1], f32, name="psum1")
            nc.tensor.matmul(psum1[:], ones_f[:], csum[:], start=True,
                             stop=True)
            partial = spool.tile([1, 1], f32, name="partial")
            nc.vector.tensor_scalar(partial[:], psum1[:], INV_PN, 0.0,
                                    Alu.mult, Alu.add)
            nc.sync.dma_start(out_d[:], partial[:])

    nc.compile()
    return nc


def kernel(y_pred, y_true, index_p, pos_idx, u_all, u_pos):
    global LAST_RESULT

    yp = np.asarray(y_pred, dtype=np.float32).reshape(-1)
    maskf = (np.asarray(y_true, dtype=np.float32).reshape(-1) == 1.0
             ).astype(np.float32)
    index_p = np.asarray(index_p).reshape(-1)
    pos_idx = np.asarray(pos_idx).reshape(-1)
    u_all_b = np.asarray(u_all, dtype=np.float32).reshape(-1)[index_p]
    u_pos_b = np.asarray(u_pos, dtype=np.float32).reshape(-1)[index_p]

    f_ps = yp[pos_idx]
    negf = (MARGIN - f_ps).astype(np.float32)       # (P,)

    nc = _COMPILED.get("nc")
    if nc is None:
        nc = _build()
        _COMPILED["nc"] = nc

    yb = np.ascontiguousarray(yp.reshape(NB, JB).T)
    maskb = np.ascontiguousarray(maskf.reshape(NB, JB).T)

    # sums of W columns over the ACT (sign) block range, for the
    # Hs -> H correction: H.W = (Hs.W + sum(W))/2
    ys = yp[DVE_BLOCKS * JB:].astype(np.float64)
    ms = maskf[DVE_BLOCKS * JB:].astype(np.float64)
    corr = np.array([ys.size, ys.sum(), (ys * ys).sum(),
                     ms.sum(), (ms * ys).sum(), (ms * ys * ys).sum()],
                    dtype=np.float64).astype(np.float32)
    corrb = np.ascontiguousarray(
        np.broadcast_to(corr, (JB, 6))).astype(np.float32)

    in_maps = []
    for c in range(N_CORES):
        rs = slice(c * PC, (c + 1) * PC)
        negf_c = negf[rs]
        in_maps.append({
            "yb": yb,
            "maskb": maskb,
            "negf": np.ascontiguousarray(
                np.broadcast_to(negf_c, (JB, PC))).astype(np.float32),
            "negfT": np.ascontiguousarray(negf_c.reshape(NH, JB).T),
            "uallT": np.ascontiguousarray(
                u_all_b[rs].reshape(NH, JB).T).astype(np.float32),
            "uposT": np.ascontiguousarray(
                u_pos_b[rs].reshape(NH, JB).T).astype(np.float32),
            "corrb": corrb,
        })

    res = bass_utils.run_bass_kernel_spmd(
        nc, in_maps, core_ids=list(range(N_CORES)), trace=TRACE)
    LAST_RESULT = res

    total = np.float32(0.0)
    for c in range(N_CORES):
        total = np.float32(total + res.results[c]["out"][0, 0])
    return np.asarray(total, dtype=np.float32)
